# revision 1
# baseline (speedup 1.0000x reference)
"""MHA on 8 NeuronCores, v2: query-token-sharded attention.

Core c owns token block c = (batch c//2, seq half c%2), 1024 tokens.
  - Phase 1 (token-parallel): Q^T, K^T, V for my block, all 1024 dims, bf16.
    Q^T never leaves SBUF. K^T and V go to pairwise AllGather (groups
    [2b, 2b+1]) so both cores of a batch hold the batch's full-sequence
    K^T [1024 d, 2048] and V [2048, 1024].
  - Phase 2: dense attention for MY 1024 queries x all 16 heads over the
    batch's 2048 keys. Scores transposed (S^T[k, q]) -> exp on ACT ->
    PV with a ones-row giving the softmax denominator for free; division
    via DRAM-broadcast reciprocal (reshaped [64, 16] so DVE reciprocal is
    cheap). Normalized A^T goes straight into SBUF tiles laid out for the
    output projection.
  - Phase 3: out = A @ wo^T for my tokens, entirely local. Host concatenates
    the 8 disjoint token blocks.

Only communication: 2 pairwise AllGathers (2MB in / 4MB out each),
fully overlapped with phase-1/2 compute. bf16 matmuls, fp32 PSUM.
"""
import numpy as np
import ml_dtypes

import concourse.bass as bass
import concourse.bacc as bacc
import concourse.tile as tile
import concourse.mybir as mybir

N_CORES = 8
P = 128
B, S, D = 4, 2048, 1024
TOK = 1024  # my tokens
CD = D // P
QB = 512
NKC = S // P  # 16 key chunks
F32 = mybir.dt.float32
BF16 = mybir.dt.bfloat16
EXP = mybir.ActivationFunctionType.Exp
PAIR_GROUPS = [[2 * i, 2 * i + 1] for i in range(4)]

_CACHE = {}


def _n_excess_waits(nc):
    import json

    m = json.loads(nc.to_json_bytes())
    insts = [i for f in m["functions"] for b in f["blocks"] for i in b["instructions"]]
    return sum(
        1
        for i in insts
        if len((i.get("sync_info") or {}).get("on_wait", [])) >= 2
        and i.get("opcode") != "EventSemaphore"
    )


def _finish(nc):
    nc.compile()
    import bass_rust

    for _ in range(6):
        if _n_excess_waits(nc) == 0:
            break
        bass_rust.generate_event_semaphores(nc)
    assert _n_excess_waits(nc) == 0, "excess sync waits remain"
    nc.codegen_inst_isa_subclasses()
    return nc


def build_nc(scopes=False):
    nc = bacc.Bacc("TRN2", target_bir_lowering=False, debug=False, num_devices=N_CORES)

    xqT_d = nc.dram_tensor("xqT", [D, TOK], BF16, kind="ExternalInput").ap()
    xkT_d = nc.dram_tensor("xkT", [D, TOK], BF16, kind="ExternalInput").ap()
    xvT_d = nc.dram_tensor("xvT", [D, TOK], BF16, kind="ExternalInput").ap()
    wqkvT = nc.dram_tensor("wqkvT", [D, 3 * D], BF16, kind="ExternalInput").ap()
    woT = nc.dram_tensor("woT", [D, D], BF16, kind="ExternalInput").ap()
    out = nc.dram_tensor("out", [TOK, D], F32, kind="ExternalOutput").ap()

    # pairwise exchange buffers
    kag_i = nc.dram_tensor("kag_i", [D, TOK], BF16).ap()
    kag_oA = nc.dram_tensor("kag_oA", [2, D // 2, TOK], BF16).ap()  # d-chunks 0-3
    kag_oB = nc.dram_tensor("kag_oB", [2, D // 2, TOK], BF16).ap()  # d-chunks 4-7
    vag_i = nc.dram_tensor("vag_i", [TOK, D], BF16).ap()
    vag_os = [
        nc.dram_tensor(f"vag_o{q}", [2, TOK // 4, D], BF16).ap() for q in range(4)
    ]
    den_d = nc.dram_tensor("den_d", [16, TOK], F32).ap()
    recip_d = nc.dram_tensor("recip_d", [16, TOK], F32).ap()

    from contextlib import ExitStack, nullcontext

    def scope(name):
        return nc.named_scope(name) if scopes else nullcontext()

    AG_KW = dict(
        kind="AllGather", op=mybir.AluOpType.bypass, replica_groups=PAIR_GROUPS
    )

    with tile.TileContext(nc) as tc:
        persist = ExitStack()
        qp = persist.enter_context(tc.tile_pool(name="qp", bufs=1))
        wop = persist.enter_context(tc.tile_pool(name="wop", bufs=1))
        ltp = persist.enter_context(tc.tile_pool(name="ltp", bufs=1))

        # ---------------- Phase 1: K, V (exchanged) then Q (stays local) ----
        with ExitStack() as ph1:
            xts = ph1.enter_context(tc.tile_pool(name="xts", bufs=1))
            wp = ph1.enter_context(tc.tile_pool(name="wp", bufs=1))
            ev1 = ph1.enter_context(tc.tile_pool(name="ev1", bufs=4))
            ps1 = ph1.enter_context(tc.tile_pool(name="ps1", bufs=3, space="PSUM"))

            with scope("load"):
                w_t, xqT, xkT, xvT = [], [], [], []
                # interleave w and xk loads so proj_k (first) starts ASAP
                for j in range(CD):
                    wt = wp.tile([P, 3 * D], BF16, name=f"w_{j}")
                    nc.sync.dma_start(out=wt, in_=wqkvT[j * P : (j + 1) * P, :])
                    w_t.append(wt)
                    t = xts.tile([P, TOK], BF16, name=f"xkT_{j}")
                    nc.sync.dma_start(out=t, in_=xkT_d[j * P : (j + 1) * P, :])
                    xkT.append(t)
                for nm, x, lst in (("v", xvT_d, xvT), ("q", xqT_d, xqT)):
                    for j in range(CD):
                        t = xts.tile([P, TOK], BF16, name=f"x{nm}T_{j}")
                        nc.sync.dma_start(out=t, in_=x[j * P : (j + 1) * P, :])
                        lst.append(t)

            # K^T [d-chunk, tok] -> kag_i
            with scope("proj_k"):
                for i in range(CD):
                    ps = ps1.tile([P, TOK], F32, name="ps_k", tag="ps1")
                    for j in range(CD):
                        lhsT = w_t[j][:, D + i * P : D + (i + 1) * P]
                        for h in range(TOK // QB):
                            nc.tensor.matmul(
                                ps[:, h * QB : (h + 1) * QB],
                                lhsT,
                                xkT[j][:, h * QB : (h + 1) * QB],
                                start=(j == 0),
                                stop=(j == CD - 1),
                            )
                    sb = ev1.tile([P, TOK], BF16, name="sb_k", tag="ev1")
                    (nc.scalar.copy if i % 2 == 0 else nc.vector.tensor_copy)(sb, ps)
                    nc.sync.dma_start(out=kag_i[i * P : (i + 1) * P, :], in_=sb)
            with scope("ag_k"):
                nc.gpsimd.collective_compute(
                    ins=[kag_i[0 : D // 2, :]], outs=[kag_oA[:]], **AG_KW
                )
                nc.gpsimd.collective_compute(
                    ins=[kag_i[D // 2 : D, :]], outs=[kag_oB[:]], **AG_KW
                )

            # Q^T [d-chunk, tok] -> SBUF (persistent)
            with scope("proj_q"):
                qT_t = []
                for i in range(CD):
                    ps = ps1.tile([P, TOK], F32, name="ps_q", tag="ps1")
                    for j in range(CD):
                        lhsT = w_t[j][:, i * P : (i + 1) * P]
                        for h in range(TOK // QB):
                            nc.tensor.matmul(
                                ps[:, h * QB : (h + 1) * QB],
                                lhsT,
                                xqT[j][:, h * QB : (h + 1) * QB],
                                start=(j == 0),
                                stop=(j == CD - 1),
                            )
                    qt = qp.tile([P, TOK], BF16, name=f"qT_{i}")
                    (nc.scalar.copy if i % 2 == 0 else nc.vector.tensor_copy)(qt, ps)
                    qT_t.append(qt)

            # V [tok-chunk, d] -> vag_i
            with scope("proj_v"):
                for t_i in range(CD):
                    ps = ps1.tile([P, D], F32, name="ps_v", tag="ps1")
                    for j in range(CD):
                        lhsT = xvT[j][:, t_i * P : (t_i + 1) * P]
                        for h in range(D // QB):
                            nc.tensor.matmul(
                                ps[:, h * QB : (h + 1) * QB],
                                lhsT,
                                w_t[j][:, 2 * D + h * QB : 2 * D + (h + 1) * QB],
                                start=(j == 0),
                                stop=(j == CD - 1),
                            )
                    sb = ev1.tile([P, D], BF16, name="sb_v", tag="ev1")
                    (nc.scalar.copy if t_i % 2 == 0 else nc.vector.tensor_copy)(sb, ps)
                    nc.sync.dma_start(out=vag_i[t_i * P : (t_i + 1) * P, :], in_=sb)
            with scope("ag_v"):
                for q in range(4):
                    nc.gpsimd.collective_compute(
                        ins=[vag_i[q * TOK // 4 : (q + 1) * TOK // 4, :]],
                        outs=[vag_os[q][:]],
                        **AG_KW,
                    )

        # ---------------- Phase 2: attention, 16 heads x my 1024 queries ----
        with ExitStack() as ph2:
            kst = ph2.enter_context(tc.tile_pool(name="kst", bufs=1))
            vp = ph2.enter_context(tc.tile_pool(name="vp", bufs=3))
            pt = ph2.enter_context(tc.tile_pool(name="pt", bufs=4))
            at = ph2.enter_context(tc.tile_pool(name="at", bufs=3))
            sm = ph2.enter_context(tc.tile_pool(name="sm", bufs=2))
            ps2 = ExitStack()
            s_ps = ps2.enter_context(tc.tile_pool(name="s_ps", bufs=2, space="PSUM"))
            pv_ps = ps2.enter_context(tc.tile_pool(name="pv_ps", bufs=2, space="PSUM"))

            # woT prefetch (phase 3) and lt output tiles
            wo_t = []
            for j in range(CD):
                wt3 = wop.tile([P, D], BF16, name=f"wo_{j}")
                nc.sync.dma_start(out=wt3, in_=woT[j * P : (j + 1) * P, :])
                wo_t.append(wt3)
            lts = [ltp.tile([P, TOK], BF16, name=f"lt_{i}") for i in range(CD)]

            # stage gathered K^T as 8 SBUF tiles [128 d-chunk, 2048 k]
            kT_s = []
            for j in range(CD):
                t = kst.tile([P, S], BF16, name=f"kTs_{j}")
                kg = kag_oA if j < 4 else kag_oB
                jj = j % 4
                nc.sync.dma_start(out=t[:, 0:TOK], in_=kg[0, jj * P : (jj + 1) * P, :])
                nc.sync.dma_start(out=t[:, TOK:S], in_=kg[1, jj * P : (jj + 1) * P, :])
                kT_s.append(t)

            KCS = [0, 1, 8, 9, 2, 3, 10, 11, 4, 5, 12, 13, 6, 7, 14, 15]
            GROUPS = [(0, 3), (3, 6), (6, 9), (9, 12), (12, 15), (15, 16)]

            # flatten (head, qblock, group) into a software pipeline with a
            # one-group scores lookahead so ACT(exp) never waits on PE latency
            units = []  # (h, qb) state
            vts, araws, pvs = {}, {}, {}

            def load_head(h):
                v_t = vp.tile([P, NKC, 65], BF16, name="v_t", tag="vp")
                for q in range(4):
                    for half in range(2):
                        vsrc = vag_os[q][half, :, 64 * h : 64 * h + 64]
                        nc.sync.dma_start(
                            out=v_t[:, 4 * q + 2 * half : 4 * q + 2 * half + 2, 0:64],
                            in_=vsrc.rearrange("(kc p) d -> p kc d", p=P),
                        )
                nc.vector.memset(v_t[:, :, 64:65], 1.0)
                vts[h] = v_t

            steps = [
                (h, qb, gi)
                for h in range(16)
                for qb in range(TOK // QB)
                for gi in range(len(GROUPS))
            ]

            def emit_scores(step):
                h, qb, gi = step
                if qb == 0 and gi == 0:
                    load_head(h)
                    araws[h] = at.tile([65, TOK], F32, name="a_raw", tag="at")
                g0, g1 = GROUPS[gi]
                if gi == 0:
                    pvs[(h, qb)] = pv_ps.tile([65, QB], F32, name="pv", tag="pv_ps")
                r = slice(64 * (h % 2), 64 * (h % 2) + 64)
                qs = slice(qb * QB, (qb + 1) * QB)
                sg = s_ps.tile([P, 3, QB], F32, name="sg", tag="s_ps")
                for pos in range(g0, g1):
                    kc = KCS[pos]
                    nc.tensor.matmul(
                        sg[:, pos - g0, :],
                        kT_s[h // 2][r, kc * P : (kc + 1) * P],
                        qT_t[h // 2][r, qs],
                        start=True,
                        stop=True,
                    )
                return sg

            def emit_exp_pv(step, sg):
                h, qb, gi = step
                g0, g1 = GROUPS[gi]
                n = g1 - g0
                pg = pt.tile([P, 3, QB], BF16, name="pg", tag="pt")
                nc.scalar.activation(pg[:, 0:n, :], sg[:, 0:n, :], EXP, scale=0.125)
                return pg

            def emit_pv(step, pg):
                h, qb, gi = step
                g0, g1 = GROUPS[gi]
                for pos in range(g0, g1):
                    nc.tensor.matmul(
                        pvs[(h, qb)],
                        vts[h][:, pos, :],
                        pg[:, pos - g0, :],
                        start=(pos == 0),
                        stop=(pos == NKC - 1),
                    )
                if g1 == NKC:
                    qs = slice(qb * QB, (qb + 1) * QB)
                    nc.vector.tensor_copy(araws[h][:, qs], pvs[(h, qb)])
                    if qb == TOK // QB - 1:
                        finish_head(h)

            def finish_head(h):
                with scope(f"norm_h{h}"):
                    a_raw = araws.pop(h)
                    nc.sync.dma_start(out=den_d[h : h + 1, :], in_=a_raw[64:65, :])
                    dsq = sm.tile([64, 16], F32, name="dsq", tag="smd")
                    nc.sync.dma_start(
                        out=dsq,
                        in_=bass.AP(
                            tensor=den_d.tensor,
                            offset=h * TOK,
                            ap=[[16, 64], [1, 16]],
                        ),
                    )
                    rsq = sm.tile([64, 16], F32, name="rsq", tag="smr")
                    nc.vector.reciprocal(rsq, dsq)
                    nc.sync.dma_start(
                        out=bass.AP(
                            tensor=recip_d.tensor,
                            offset=h * TOK,
                            ap=[[16, 64], [1, 16]],
                        ),
                        in_=rsq,
                    )
                    bc = at.tile([64, TOK], F32, name="bc", tag="at2")
                    nc.sync.dma_start(
                        out=bc,
                        in_=bass.AP(
                            tensor=recip_d.tensor,
                            offset=h * TOK,
                            ap=[[0, 64], [1, TOK]],
                        ),
                    )
                    rr = slice(64 * (h % 2), 64 * (h % 2) + 64)
                    nc.vector.tensor_mul(lts[h // 2][rr, :], a_raw[0:64, :], bc)

            with scope("attn"):
                sg_cur = emit_scores(steps[0])
                for i, step in enumerate(steps):
                    pg = emit_exp_pv(step, sg_cur)
                    if i + 1 < len(steps):
                        sg_cur = emit_scores(steps[i + 1])
                    emit_pv(step, pg)

            ps2.close()

            # ---------------- Phase 3: output projection (local) -------------
            with scope("wo"):
                ev3 = ph2.enter_context(tc.tile_pool(name="ev3", bufs=3))
                ps3p = ph2.enter_context(tc.tile_pool(name="ps3p", bufs=3, space="PSUM"))
                for t_i in range(CD):
                    ps3 = ps3p.tile([P, D], F32, name="ps3", tag="ps3")
                    for sc in range(CD):
                        for hh in range(2):
                            nc.tensor.matmul(
                                ps3[:, hh * QB : (hh + 1) * QB],
                                lts[sc][:, t_i * P : (t_i + 1) * P],
                                wo_t[sc][:, hh * QB : (hh + 1) * QB],
                                start=(sc == 0),
                                stop=(sc == CD - 1),
                            )
                    ob = ev3.tile([P, D], F32, name="ob", tag="ev3")
                    nc.vector.tensor_copy(ob, ps3)
                    nc.sync.dma_start(out=out[t_i * P : (t_i + 1) * P, :], in_=ob)

        persist.close()

    return _finish(nc)


def _get_nc(scopes=False):
    key = ("nc", scopes)
    if key not in _CACHE:
        _CACHE[key] = build_nc(scopes)
    return _CACHE[key]


def make_in_maps(query, key, value, wq, wk, wv, wo):
    qf = np.asarray(query, np.float32).reshape(B * S, D)
    kf = np.asarray(key, np.float32).reshape(B * S, D)
    vf = np.asarray(value, np.float32).reshape(B * S, D)
    wqkvT = np.ascontiguousarray(
        np.concatenate([np.asarray(wq), np.asarray(wk), np.asarray(wv)], 0).T
    ).astype(ml_dtypes.bfloat16)
    woT_h = np.ascontiguousarray(np.asarray(wo).T).astype(ml_dtypes.bfloat16)
    in_maps = []
    for c in range(N_CORES):
        sl = slice(c * TOK, (c + 1) * TOK)
        in_maps.append(
            {
                "xqT": np.ascontiguousarray(qf[sl].T).astype(ml_dtypes.bfloat16),
                "xkT": np.ascontiguousarray(kf[sl].T).astype(ml_dtypes.bfloat16),
                "xvT": np.ascontiguousarray(vf[sl].T).astype(ml_dtypes.bfloat16),
                "wqkvT": wqkvT,
                "woT": woT_h,
            }
        )
    return in_maps


def assemble(results):
    blocks = [results[c]["out"] for c in range(N_CORES)]
    return np.concatenate(blocks, 0).reshape(B, S, D).astype(np.float32)


def kernel(query, key, value, mask, wq, wk, wv, wo):
    # mask is all-False in this problem: softmax without masking.
    nc = _get_nc()
    in_maps = make_in_maps(query, key, value, wq, wk, wv, wo)
    from concourse.bass_utils import run_bass_kernel_spmd

    res = run_bass_kernel_spmd(nc, in_maps, list(range(N_CORES)))
    return assemble(res.results)



# revision 7
# speedup vs baseline: 1.2177x; 1.2177x over previous
"""MHA on 8 NeuronCores, v3: query-token-sharded attention, ACT-bound pipeline.

Core c owns token block c = (batch c//2, seq half c%2), 1024 tokens.
The attention phase is ACT(exp)-bound (~262k exp-rows/core is the hard
floor), so everything is scheduled around keeping ACT 100% busy:
  - Phase order: warmup spin (HAM un-throttle) -> proj_v t0-3 -> proj_k ->
    proj_v t4-7 -> proj_q ch0 -> attention with proj_q ch1-7 interleaved as
    PE filler -> wo.
  - Dummy collective at t~0 absorbs the one-time CC rendezvous barrier.
    CC order (deadline-sorted): V-half0, K-halfA, V-half1, K-halfB.
  - Attention step = (head, kc): one scores matmul (n=1024), one exp
    [128,1024], two PV matmuls (n=512) lagged L_PV steps behind scores.
  - PSUM: sg tag 2x[128,1024] (4 banks) + proj tag 1x[128,1024] (2) +
    pv tag 2x[65,512] (2) = 8 banks.
  - wk/wq are consumed one 128-column chunk at a time, so the host passes
    them chunk-major ([8, 128, 1024]) and they stream through small tiles.
  - Softmax denominator from the ones-column of V (65th col); pv psum is
    released via one DVE copy to a_raw; normalization (DVE reciprocal +
    GpSimd partition_broadcast + DVE mul) runs off the critical path.
"""
import numpy as np
import ml_dtypes

import concourse.bass as bass
import concourse.bacc as bacc
import concourse.tile as tile
import concourse.mybir as mybir

N_CORES = 8
P = 128
B, S, D = 4, 2048, 1024
TOK = 1024  # my tokens
CD = D // P  # 8 chunks
NKC = S // P  # 16 key chunks
F32 = mybir.dt.float32
BF16 = mybir.dt.bfloat16
EXP = mybir.ActivationFunctionType.Exp
PAIR_GROUPS = [[2 * i, 2 * i + 1] for i in range(4)]

# processing order of key chunks: V-arrival order (half0 both slots, then half1)
KC_ORDER = [0, 1, 2, 3, 8, 9, 10, 11, 4, 5, 6, 7, 12, 13, 14, 15]
L_PV = 6  # PV lag in steps behind scores/exp

_CACHE = {}


def _n_excess_waits(nc):
    import json

    m = json.loads(nc.to_json_bytes())
    insts = [i for f in m["functions"] for b in f["blocks"] for i in b["instructions"]]
    return sum(
        1
        for i in insts
        if len((i.get("sync_info") or {}).get("on_wait", [])) >= 2
        and i.get("opcode") != "EventSemaphore"
    )


def _finish(nc):
    nc.compile()
    import bass_rust

    for _ in range(6):
        if _n_excess_waits(nc) == 0:
            break
        bass_rust.generate_event_semaphores(nc)
    assert _n_excess_waits(nc) == 0, "excess sync waits remain"
    nc.codegen_inst_isa_subclasses()
    return nc


def build_nc(scopes=False):
    nc = bacc.Bacc("TRN2", target_bir_lowering=False, debug=False, num_devices=N_CORES)

    xqT_d = nc.dram_tensor("xqT", [D, TOK], BF16, kind="ExternalInput").ap()
    xkT_d = nc.dram_tensor("xkT", [D, TOK], BF16, kind="ExternalInput").ap()
    xvT_d = nc.dram_tensor("xvT", [D, TOK], BF16, kind="ExternalInput").ap()
    # wk/wq chunk-major: [out-chunk i, partition p, (j, q)] with
    # wk4[i, p, j*128+q] = wk.T[j*128+p, i*128+q]
    wk4_d = nc.dram_tensor("wk4", [CD, P, D], BF16, kind="ExternalInput").ap()
    wq4_d = nc.dram_tensor("wq4", [CD, P, D], BF16, kind="ExternalInput").ap()
    wvT_d = nc.dram_tensor("wvT", [D, D], BF16, kind="ExternalInput").ap()
    woT_d = nc.dram_tensor("woT", [D, D], BF16, kind="ExternalInput").ap()
    out = nc.dram_tensor("out", [TOK, D], F32, kind="ExternalOutput").ap()

    # exchange buffers
    dum_i = nc.dram_tensor("dum_i", [1, 16], BF16).ap()
    dum_o = nc.dram_tensor("dum_o", [2, 1, 16], BF16).ap()
    kag_i = nc.dram_tensor("kag_i", [D, TOK], BF16).ap()
    kag_oA = nc.dram_tensor("kag_oA", [2, D // 2, TOK], BF16).ap()  # d-chunks 0-3
    kag_oB = nc.dram_tensor("kag_oB", [2, D // 2, TOK], BF16).ap()  # d-chunks 4-7
    vag_i = nc.dram_tensor("vag_i", [TOK, D], BF16).ap()
    # V halves by my-token halves: vag_os[half][slot, tok512, d]
    vag_os = [
        nc.dram_tensor(f"vag_o{h}", [2, TOK // 2, D], BF16).ap() for h in range(2)
    ]

    from contextlib import ExitStack, nullcontext

    def scope(name):
        return nc.named_scope(name) if scopes else nullcontext()

    AG_KW = dict(
        kind="AllGather", op=mybir.AluOpType.bypass, replica_groups=PAIR_GROUPS
    )

    with tile.TileContext(nc) as tc:
        persist = ExitStack()
        qp = persist.enter_context(tc.tile_pool(name="qp", bufs=1))
        ltp = persist.enter_context(tc.tile_pool(name="ltp", bufs=1))
        kst = persist.enter_context(tc.tile_pool(name="kst", bufs=1))
        psp = persist.enter_context(tc.tile_pool(name="psp", bufs=1, space="PSUM"))
        stp = persist.enter_context(tc.tile_pool(name="stp", bufs=1))
        vp = persist.enter_context(tc.tile_pool(name="vp", bufs=1))
        pgp = persist.enter_context(tc.tile_pool(name="pgp", bufs=1))
        smp = persist.enter_context(tc.tile_pool(name="smp", bufs=1))
        arp = persist.enter_context(tc.tile_pool(name="arp", bufs=1))

        # close order is wkx -> wvx -> wqx, so create in reverse (pool stack is LIFO)
        wqx_stack = ExitStack()
        wqx = wqx_stack.enter_context(tc.tile_pool(name="wqx", bufs=1))
        wvx_stack = ExitStack()
        wvx = wvx_stack.enter_context(tc.tile_pool(name="wvx", bufs=1))
        wkx_stack = ExitStack()
        wkx = wkx_stack.enter_context(tc.tile_pool(name="wkx", bufs=1))

        # ---------------- t~0: dummy collective to absorb CC barrier --------
        with scope("warm"):
            wtile = smp.tile([128, 512], BF16, name="wtile")
            nc.vector.memset(wtile, 0.001)
            dumt = smp.tile([1, 16], BF16, name="dumt")
            nc.vector.memset(dumt, 0.0)
            nc.sync.dma_start(out=dum_i, in_=dumt)
            nc.gpsimd.collective_compute(ins=[dum_i], outs=[dum_o], **AG_KW)

        # ---------------- loads (order matters: v, k, q) --------------------
        wk_c = {}  # streamed wk chunk tiles: i -> [128, 8, 128]
        wq_c = {}

        def load_wk_chunk(i):
            t = wkx.tile([P, CD, P], BF16, name=f"wkc_{i}", tag="wkc", bufs=3)
            nc.sync.dma_start(out=t, in_=wk4_d[i].rearrange("p (j q) -> p j q", q=P))
            wk_c[i] = t

        def load_wq_chunk(i):
            t = wqx.tile([P, CD, P], BF16, name=f"wqc_{i}", tag="wqc", bufs=3)
            nc.sync.dma_start(out=t, in_=wq4_d[i].rearrange("p (j q) -> p j q", q=P))
            wq_c[i] = t

        with scope("load"):
            xvT, wv_t, xkT, xqT = [], [], [], []
            for j in range(CD):
                t = wvx.tile([P, TOK], BF16, name=f"xvT_{j}")
                nc.sync.dma_start(out=t, in_=xvT_d[j * P : (j + 1) * P, :])
                xvT.append(t)
                w = wvx.tile([P, D], BF16, name=f"wv_{j}")
                nc.sync.dma_start(out=w, in_=wvT_d[j * P : (j + 1) * P, :])
                wv_t.append(w)
            for j in range(CD):
                t = wkx.tile([P, TOK], BF16, name=f"xkT_{j}")
                nc.sync.dma_start(out=t, in_=xkT_d[j * P : (j + 1) * P, :])
                xkT.append(t)
            load_wk_chunk(0)
            load_wk_chunk(1)
            for j in range(CD):
                t = wqx.tile([P, TOK], BF16, name=f"xqT_{j}")
                nc.sync.dma_start(out=t, in_=xqT_d[j * P : (j + 1) * P, :])
                xqT.append(t)
            load_wq_chunk(0)

        def mm2(ps, lhsT, rhs, start, stop):
            # ISA caps matmul output at 512 fp32 elements (one PSUM bank):
            # emit two 512-wide matmuls covering a [*, 1024] psum tile
            for hh in range(2):
                nc.tensor.matmul(
                    ps[:, hh * 512 : (hh + 1) * 512],
                    lhsT,
                    rhs[:, hh * 512 : (hh + 1) * 512],
                    start=start,
                    stop=stop,
                )

        # ---------------- warmup spin: un-throttle PE HAM early -------------
        with scope("spin"):
            for k in range(24):
                ps = psp.tile([P, TOK], F32, name="wm", tag="sg", bufs=2)
                nc.tensor.matmul(
                    ps[:, 0:512], wtile[:, 0:128], wtile, start=True, stop=True
                )

        # ---------------- phase 1a: proj_v t0-3 -----------------------------
        def proj_v_chunk(t_i, copy_eng):
            ps = psp.tile([P, D], F32, name="ps_v", tag="sg", bufs=2)
            for j in range(CD):
                mm2(ps, xvT[j][:, t_i * P : (t_i + 1) * P], wv_t[j],
                    start=(j == 0), stop=(j == CD - 1))
            sb = stp.tile([P, D], BF16, name="sb_v", tag="st", bufs=3)
            copy_eng(sb, ps)
            nc.sync.dma_start(out=vag_i[t_i * P : (t_i + 1) * P, :], in_=sb)

        with scope("proj_v_a"):
            for t_i in range(4):
                proj_v_chunk(
                    t_i, nc.scalar.copy if t_i % 2 == 0 else nc.vector.tensor_copy
                )
        with scope("ag_v0"):
            nc.gpsimd.collective_compute(
                ins=[vag_i[0 : TOK // 2, :]], outs=[vag_os[0][:]], **AG_KW
            )

        # ---------------- phase 1b: proj_k ----------------------------------
        def proj_k_chunk(i, copy_eng):
            if i + 2 < CD:
                load_wk_chunk(i + 2)
            ps = psp.tile([P, TOK], F32, name="ps_k", tag="sg", bufs=2)
            wkc = wk_c.pop(i)
            for j in range(CD):
                mm2(ps, wkc[:, j, :], xkT[j], start=(j == 0), stop=(j == CD - 1))
            sb = stp.tile([P, TOK], BF16, name="sb_k", tag="st", bufs=3)
            copy_eng(sb, ps)
            nc.sync.dma_start(out=kag_i[i * P : (i + 1) * P, :], in_=sb)

        with scope("proj_k"):
            for i in range(CD):
                proj_k_chunk(i, nc.scalar.copy if i % 2 == 0 else nc.vector.tensor_copy)
                if i == 3:
                    with scope("ag_kA"):
                        nc.gpsimd.collective_compute(
                            ins=[kag_i[0 : D // 2, :]], outs=[kag_oA[:]], **AG_KW
                        )
        wkx_stack.close()

        # ---------------- phase 1c: proj_v t4-7, then V1 + KB ags -----------
        with scope("proj_v_b"):
            for t_i in range(4, 8):
                proj_v_chunk(
                    t_i, nc.scalar.copy if t_i % 2 == 0 else nc.vector.tensor_copy
                )
        with scope("ag_v1"):
            nc.gpsimd.collective_compute(
                ins=[vag_i[TOK // 2 : TOK, :]], outs=[vag_os[1][:]], **AG_KW
            )
        with scope("ag_kB"):
            nc.gpsimd.collective_compute(
                ins=[kag_i[D // 2 : D, :]], outs=[kag_oB[:]], **AG_KW
            )
        wvx_stack.close()

        # ---------------- K^T staging into SBUF -----------------------------
        kT_s = []
        with scope("kstage"):
            for j in range(CD):
                t = kst.tile([P, S], BF16, name=f"kTs_{j}")
                kg = kag_oA if j < 4 else kag_oB
                jj = j % 4
                nc.sync.dma_start(out=t[:, 0:TOK], in_=kg[0, jj * P : (jj + 1) * P, :])
                nc.sync.dma_start(out=t[:, TOK:S], in_=kg[1, jj * P : (jj + 1) * P, :])
                kT_s.append(t)

        # ---------------- proj_q ch0 (rest are attn filler) -----------------
        qT_t = [None] * CD

        with scope("proj_q0"):
            load_wq_chunk(1)
            ps0 = psp.tile([P, TOK], F32, name="ps_q0", tag="sg", bufs=2)
            wqc0 = wq_c.pop(0)
            for j in range(CD):
                mm2(ps0, wqc0[:, j, :], xqT[j], start=(j == 0), stop=(j == CD - 1))
            qt0 = qp.tile([P, TOK], BF16, name="qT_0")
            nc.scalar.copy(qt0, ps0)
            qT_t[0] = qt0

        lts = [ltp.tile([P, TOK], BF16, name=f"lt_{i}") for i in range(CD)]

        # ---------------- attention ----------------------------------------
        filler_state = {}
        filler = []
        for i in range(1, CD):
            for j in range(CD):
                def mk(i=i, j=j):
                    def emit():
                        if j == 0:
                            if i + 1 < CD:
                                load_wq_chunk(i + 1)
                            filler_state["ps"] = psp.tile(
                                [P, TOK], F32, name="ps_qf", tag="proj", bufs=1
                            )
                            filler_state["w"] = wq_c.pop(i)
                        mm2(filler_state["ps"], filler_state["w"][:, j, :],
                            xqT[j], start=(j == 0), stop=(j == CD - 1))
                        if j == CD - 1:
                            qt = qp.tile([P, TOK], BF16, name=f"qT_{i}")
                            nc.vector.tensor_copy(qt, filler_state["ps"])
                            qT_t[i] = qt
                    return emit
                filler.append(mk())
        filler.reverse()  # so filler.pop() yields chunk 1 first

        steps = [(h, pos) for h in range(16) for pos in range(NKC)]
        sgs, pgs, pvs, vts = {}, {}, {}, {}

        def load_head(h):
            v_t = vp.tile([P, NKC, 65], BF16, name="v_t", tag="vp", bufs=3)
            for half in range(2):
                for slot in range(2):
                    vsrc = vag_os[half][slot, :, 64 * h : 64 * h + 64]
                    base = 8 * half + 4 * slot
                    nc.sync.dma_start(
                        out=v_t[:, base : base + 4, 0:64],
                        in_=vsrc.rearrange("(kc p) d -> p kc d", p=P),
                    )
            nc.gpsimd.memset(v_t[:, :, 64:65], 1.0)
            vts[h] = v_t
            # vts pos p: half=p//8, slot=(p%8)//4, i=p%4 -> kc = slot*8+half*4+i
            # which equals KC_ORDER[p].

        def emit_scores(s):
            h, pos = steps[s]
            if pos == 0:
                load_head(h)
            kc = KC_ORDER[pos]
            r = slice(64 * (h % 2), 64 * (h % 2) + 64)
            sg = psp.tile([P, TOK], F32, name="sg", tag="sg", bufs=2)
            mm2(sg, kT_s[h // 2][r, kc * P : (kc + 1) * P], qT_t[h // 2][r, :],
                start=True, stop=True)
            sgs[s] = sg

        def emit_exp(s):
            pg = pgp.tile([P, TOK], BF16, name="pg", tag="pg", bufs=L_PV + 2)
            nc.scalar.activation(pg, sgs.pop(s), EXP, scale=0.125)
            pgs[s] = pg

        def emit_pv(s):
            h, pos = steps[s]
            pg = pgs.pop(s)
            if pos == 0:
                pvs[(h, 0)] = psp.tile([65, 512], F32, name="pv0", tag="pv", bufs=2)
                pvs[(h, 1)] = psp.tile([65, 512], F32, name="pv1", tag="pv", bufs=2)
            for qb in range(2):
                nc.tensor.matmul(
                    pvs[(h, qb)],
                    vts[h][:, pos, :],
                    pg[:, qb * 512 : (qb + 1) * 512],
                    start=(pos == 0),
                    stop=(pos == NKC - 1),
                )
            if pos == NKC - 1:
                del vts[h]
                finish_head(h)

        def finish_head(h):
            with scope(f"norm_h{h}"):
                rr = slice(64 * (h % 2), 64 * (h % 2) + 64)
                for qb in range(2):
                    pv = pvs.pop((h, qb))
                    # one DVE copy releases the pv psum bank fast
                    ar = arp.tile([65, 512], F32, name="ar", tag="ar", bufs=2)
                    nc.vector.tensor_copy(ar, pv)
                    rsq = smp.tile([1, 512], F32, name="rsq", tag="rsq", bufs=2)
                    nc.vector.reciprocal(rsq, ar[64:65, :])
                    bc = smp.tile([64, 512], F32, name="bc", tag="bc", bufs=2)
                    nc.gpsimd.partition_broadcast(bc, rsq)
                    nc.vector.tensor_mul(
                        lts[h // 2][rr, qb * 512 : (qb + 1) * 512], ar[0:64, :], bc
                    )

        wo_t = []
        with scope("attn"):
            emit_scores(0)
            n = len(steps)
            for s in range(n):
                emit_exp(s)
                if filler:
                    filler.pop()()
                elif s == 57:
                    # filler drained: free xq/wq space, start wo prefetch there
                    wqx_stack.close()
                    wop = persist.enter_context(tc.tile_pool(name="wop", bufs=1))
                    for j in range(CD):
                        wt3 = wop.tile([P, D], BF16, name=f"wo_{j}")
                        nc.sync.dma_start(out=wt3, in_=woT_d[j * P : (j + 1) * P, :])
                        wo_t.append(wt3)
                if s + 1 < n:
                    emit_scores(s + 1)
                if s >= L_PV:
                    emit_pv(s - L_PV)
            for s in range(n - L_PV, n):
                emit_pv(s)

        # ---------------- phase 3: output projection ------------------------
        with scope("wo"):
            for t_i in range(CD):
                ps3 = psp.tile([P, D], F32, name="ps3", tag="sg", bufs=2)
                for sc in range(CD):
                    mm2(ps3, lts[sc][:, t_i * P : (t_i + 1) * P], wo_t[sc],
                        start=(sc == 0), stop=(sc == CD - 1))
                ob = stp.tile([P, D], F32, name="ob", tag="ob", bufs=2)
                nc.vector.tensor_copy(ob, ps3)
                nc.sync.dma_start(out=out[t_i * P : (t_i + 1) * P, :], in_=ob)

        persist.close()

    return _finish(nc)


def _get_nc(scopes=False):
    key = ("nc", scopes)
    if key not in _CACHE:
        _CACHE[key] = build_nc(scopes)
    return _CACHE[key]


def _chunk_major(wT):
    # wT: [D, D] = w.T ; return [CD, P, D] with out[i, p, j*128+q] = wT[j*128+p, i*128+q]
    return np.ascontiguousarray(
        wT.reshape(CD, P, CD, P).transpose(2, 1, 0, 3).reshape(CD, P, D)
    )


def make_in_maps(query, key, value, wq, wk, wv, wo):
    qf = np.asarray(query, np.float32).reshape(B * S, D)
    kf = np.asarray(key, np.float32).reshape(B * S, D)
    vf = np.asarray(value, np.float32).reshape(B * S, D)
    wk4_h = _chunk_major(np.asarray(wk).T.astype(np.float32)).astype(ml_dtypes.bfloat16)
    wq4_h = _chunk_major(np.asarray(wq).T.astype(np.float32)).astype(ml_dtypes.bfloat16)
    wvT_h = np.ascontiguousarray(np.asarray(wv).T).astype(ml_dtypes.bfloat16)
    woT_h = np.ascontiguousarray(np.asarray(wo).T).astype(ml_dtypes.bfloat16)
    in_maps = []
    for c in range(N_CORES):
        sl = slice(c * TOK, (c + 1) * TOK)
        in_maps.append(
            {
                "xqT": np.ascontiguousarray(qf[sl].T).astype(ml_dtypes.bfloat16),
                "xkT": np.ascontiguousarray(kf[sl].T).astype(ml_dtypes.bfloat16),
                "xvT": np.ascontiguousarray(vf[sl].T).astype(ml_dtypes.bfloat16),
                "wk4": wk4_h,
                "wq4": wq4_h,
                "wvT": wvT_h,
                "woT": woT_h,
            }
        )
    return in_maps


def assemble(results):
    blocks = [results[c]["out"] for c in range(N_CORES)]
    return np.concatenate(blocks, 0).reshape(B, S, D).astype(np.float32)


def kernel(query, key, value, mask, wq, wk, wv, wo):
    # mask is all-False in this problem: softmax without masking.
    nc = _get_nc()
    in_maps = make_in_maps(query, key, value, wq, wk, wv, wo)
    from concourse.bass_utils import run_bass_kernel_spmd

    res = run_bass_kernel_spmd(nc, in_maps, list(range(N_CORES)))
    return assemble(res.results)
